# revision 30
# baseline (speedup 1.0000x reference)
"""Trainium2 Bass kernel for the emoji-box decoder problem (v2).

Math: softmax(-d2) over emoji pixels is separable (rows x cols).  Per
core (64 canvas rows x 256 cols x 3ch of one picture):

  E = exp(-D2),  D2 built by ONE PE matmul with lhsT=[1, p-32, (p-32)^2]
  and rhs rows [u^2; 2*beta*u; beta^2] (u = gamma - n), giving
    E[:, 0:128]   = erT[i, (h,r)]  (row kernel, transposed, h-duplicated)
    E[:, 128:384] = ecT[j, c]      (col kernel, transposed)
  T1_ch[j, r] = sum_i img[ch,i,j] * erT[i,r]          (3 matmuls)
  U[(ch,r), c] = sum_j T1[j,(ch,r)] * ecT[j,c]        (2 matmuls)
  res = U * srow[(ch,r)] * SCOL[c] + Qv               (2 DVE ops/piece)
  with srow = rzr*rowin (transposed via PE), SCOL = ones (x) (rzc*colin),
  Qv = valid - rowin (x) colin (PE outer products).

No max-shift in the softmax: in-box distances are <= ~16 so exp(-d2)
stays in range; fully-underflowed rows are saved by a 1e-30 clamp on
the denominator and are masked anyway.

The final blend + output DMAs run OUTSIDE the TileContext so the fixed
NRT postamble (a ~6us per-engine semaphore-clear storm, longest on the
Tensor engine) overlaps the output-DMA tail instead of following it.

Sharding: 8 cores = 2 pictures x 4 row-blocks of 64 canvas rows; images
replicated; xmeta = [X row (19), r0].
"""

import sys

import numpy as np

if "/opt/trn_rl_repo" not in sys.path:
    sys.path.insert(0, "/opt/trn_rl_repo")

import concourse.bacc as bacc
import concourse.bass as bass
import concourse.mybir as mybir
import concourse.tile as tile
from concourse.bass_utils import run_bass_kernel_spmd


def _ensure_ntff_hook():
    """The image's antenv package lacks axon_hooks, so trn_boot's NTFF
    profile hook install degrades silently and run_bass_kernel_spmd
    crashes on `from antenv.axon_hooks import ...` when trace=True.
    Provide the module and install the ctypes hook ourselves."""
    import types

    try:
        from antenv.axon_hooks import get_axon_ntff_profile_hook  # noqa: F401

        return
    except ImportError:
        pass
    mod = types.ModuleType("antenv.axon_hooks")
    _hook = [None]
    mod.set_axon_ntff_profile_hook = lambda h: _hook.__setitem__(0, h)
    mod.get_axon_ntff_profile_hook = lambda: _hook[0]
    try:
        import antenv

        sys.modules["antenv.axon_hooks"] = mod
        antenv.axon_hooks = mod
        from trn_agent_boot.trn_boot import _ntff_profile_via_ctypes

        hook = _ntff_profile_via_ctypes("/opt/axon/libaxon_pjrt.so")
        if hook is not None:
            mod.set_axon_ntff_profile_hook(hook)
    except Exception:
        pass


_ensure_ntff_hook()

F32 = mybir.dt.float32
BF = mybir.dt.bfloat16
I32 = mybir.dt.int32
U32 = mybir.dt.uint32
AF = mybir.ActivationFunctionType
OP = mybir.AluOpType
AX = mybir.AxisListType

MAGIC = 8388608.0  # 2**23; x + MAGIC - MAGIC == rint(x) for 0 <= x < 2**22

N_CORES = 8
H = 256
S = 64
N_IMG = 14
RB = 64  # canvas rows per core


def build_nc():
    nc = bacc.Bacc("TRN2", target_bir_lowering=False, debug=False)

    xmeta_d = nc.dram_tensor("xmeta", [1, 20], F32, kind="ExternalInput")
    images_d = nc.dram_tensor("images", [N_IMG, 4, S, S], F32, kind="ExternalInput")
    out_d = nc.dram_tensor("out", [3, RB, H], F32, kind="ExternalOutput")

    # ---- raw allocations (persist across the TileContext boundary) ----
    sb = lambda name, shape, dt=F32: nc.alloc_sbuf_tensor(name, shape, dt)
    xrow = sb("xrow", [1, 20])
    iota256f = sb("iota256f", [1, 256])
    iotadupf = sb("iotadupf", [1, 128])      # [0..63, 0..63]
    iota_m32_row = sb("iota_m32_row", [1, 64])   # p-32 as a row (acol matmul)
    ones64_row = sb("ones64_row", [1, 64])
    iota_dup_all = sb("iota_dup_all", [64, 128])   # [0..63,0..63] every partition
    iota256_all = sb("iota256_all", [64, 256])     # 0..255 every partition
    ones_col64_bf = sb("ones_col64_bf", [64, 1], BF)
    ones128_row_bf = sb("ones128_row_bf", [1, 128], BF)
    ones256_bf = sb("ones256_bf", [1, 256], BF)
    one11_bf = sb("one11_bf", [1, 1], BF)
    blk = sb("blk", [1, 12])                 # p0 scalars
    cs = sb("cs", [1, 4])
    box2 = sb("box2", [1, 2])
    beta2 = sb("beta2", [1, 2])
    v10 = sb("v10", [1, 10])
    tmp11 = sb("tmp11", [1, 2])
    acol = sb("acol", [64, 2])               # a_r, a_c per-partition
    Dsb = sb("Dsb", [64, 384])
    D2sb = sb("D2sb", [64, 384])
    E = sb("E", [64, 384])
    Ebf = sb("Ebf", [64, 384], BF)
    wimg = sb("wimg", [64, 3, 64])
    wimg_bf = sb("wimg_bf", [64, 3, 64], BF)
    T1sb = sb("T1sb", [64, 192], BF)
    mrow = sb("mrow", [1, 128])
    mcol = sb("mcol", [1, 256])
    mrow_bf = sb("mrow_bf", [1, 128], BF)
    mtmp = sb("mtmp", [1, 512])
    rcprs = sb("rcprs", [128, 1])
    ztc = sb("ztc", [128, 1])
    zc2t = sb("zc2t", [128, 2])
    rcp2 = sb("rcp2", [128, 2])
    id128 = sb("id128", [128, 128], BF)
    ones128sq = sb("ones128sq", [128, 128], BF)
    rcp2_bf = sb("rcp2_bf", [128, 2], BF)
    rsqr = sb("rsqr", [1, 256])              # rsqrt(zrow)
    rzc = sb("rzc", [1, 256])
    scol = sb("scol", [1, 256])
    scol_bf = sb("scol_bf", [1, 256], BF)
    Uab_sb = sb("Uab_sb", [128, 256])
    Uc_sb = sb("Uc_sb", [64, 256])
    ssb = sb("ssb", [128, 1])
    vrow_bf = sb("vrow_bf", [1, 128], BF)
    negcol_bf = sb("negcol_bf", [1, 256], BF)
    mx8 = sb("mx8", [1, 8])
    idx8 = sb("idx8", [1, 8], U32)
    res_ab = sb("res_ab", [128, 256])
    res_c = sb("res_c", [64, 256])
    eps128 = sb("eps128", [128, 1])
    eps1 = sb("eps1", [1, 1])
    warm1 = sb("warm1", [1, 1])
    warm2 = sb("warm2", [1, 1])

    ps = lambda name, shape: nc.alloc_psum_tensor(name, shape)
    zb = ps("zb", [128, 512])
    zc2_ps = ps("zc2_ps", [128, 512])  # zc cols [128,0:2]; rcp rows [0:1,4:260]
    acol_ps = ps("acol_ps", [64, 2])     # zcol [128,0:1], rowincol [128,1:2], zrow [0:1,2:258]
    T1ps = ps("T1ps", [64, 192])
    Uab_ps = ps("Uab_ps", [128, 256])
    Uc_ps = ps("Uc_ps", [64, 256])
    scol_ps = ps("scol_ps", [128, 256])
    Qv_ps = ps("Qv_ps", [128, 256])

    with tile.TileContext(nc) as tc:  # noqa: F841
        # ---- warm the scalar-engine Exp table early (overlaps input DMA)
        nc.gpsimd.memset(warm1[:], 0.0)
        nc.scalar.activation(warm2[:], warm1[:], AF.Exp)

        # ---- input DMA first
        nc.sync.dma_start(xrow[:], xmeta_d[:])

        # ---- emoji index first: top-1 via Max8 + MaxIndex, straight off xrow
        nc.vector.max(mx8[:], xrow[0:1, 5:19])
        nc.vector.max_index(idx8[:], mx8[:], xrow[0:1, 5:19])
        with nc.gpsimd.register("ridx") as ridx:
            nc.gpsimd.reg_load(ridx, idx8[0:1, 0:1])
            off = nc.gpsimd.snap(ridx)
            nc.gpsimd.dma_start(
                wimg_bf[:, 0:2, :],
                images_d[bass.ds(off, 1), 0:2, :, :].squeeze(0).transpose([1, 0, 2]),
            )
        with nc.sync.register("ridx2") as ridx2:
            nc.sync.reg_load(ridx2, idx8[0:1, 0:1])
            off2 = nc.sync.snap(ridx2)
            nc.sync.dma_start(
                wimg[:, 2, :],
                images_d[bass.ds(off2, 1), 2, :, :].squeeze(0),
            )

        # ---- constants (no data deps): f32 iotas are exact for values < 2^24
        IOTA = dict(allow_small_or_imprecise_dtypes=True)
        nc.gpsimd.iota(iota256f[0:1, :], pattern=[[1, 256]], base=0, channel_multiplier=0, **IOTA)
        nc.gpsimd.iota(iotadupf[0:1, :], pattern=[[0, 2], [1, 64]], base=0, channel_multiplier=0, **IOTA)
        nc.gpsimd.iota(iota_m32_row[0:1, :], pattern=[[1, 64]], base=-32, channel_multiplier=0, **IOTA)
        nc.gpsimd.iota(iota_dup_all[:, :], pattern=[[0, 2], [1, 64]], base=0, channel_multiplier=0, **IOTA)
        nc.gpsimd.iota(iota256_all[:, :], pattern=[[1, 256]], base=0, channel_multiplier=0, **IOTA)
        nc.vector.memset(ones64_row[:], 1.0)
        nc.gpsimd.memset(ones_col64_bf[:], 1.0)
        nc.vector.memset(ones128_row_bf[:], 1.0)
        nc.vector.memset(ones256_bf[:], 1.0)
        nc.vector.memset(one11_bf[:], 1.0)
        nc.vector.memset(eps128[:], 1e-30)
        nc.gpsimd.memset(ones128sq[:], 1.0)
        nc.gpsimd.affine_select(
            id128[:], ones128sq[:], pattern=[[1, 128]], compare_op=OP.is_equal,
            fill=0.0, base=0, channel_multiplier=-1,
        )

        # ---- p0 scalar chain -> blk = [gr, gc, br, bc, x1r, x2rp, y1, y2, valid]
        nc.vector.tensor_scalar(cs[:], xrow[0:1, 0:4], 256.0, MAGIC, OP.mult, OP.add)
        nc.vector.tensor_scalar(cs[:], cs[:], MAGIC, None, OP.subtract)
        nc.vector.tensor_tensor(box2[:], cs[0:1, 1:4:2], cs[0:1, 0:3:2], OP.subtract)
        nc.vector.tensor_scalar(beta2[:], box2[:], 1.0 / 64.0, None, OP.mult)
        nc.vector.tensor_tensor(blk[0:1, 4:5], cs[0:1, 0:1], xrow[0:1, 19:20], OP.subtract)
        nc.vector.scalar_tensor_tensor(
            blk[0:1, 0:1], beta2[0:1, 0:1], 32.0, blk[0:1, 4:5], OP.mult, OP.add
        )
        nc.vector.scalar_tensor_tensor(
            blk[0:1, 1:2], beta2[0:1, 1:2], 32.0, cs[0:1, 2:3], OP.mult, OP.add
        )
        # ---- acol[p, :] = [gamma_r, gamma_c] + (p-32)*[beta_r, beta_c]
        # via two accumulating rank-1 PE matmuls (PE is idle this early)
        nc.tensor.matmul(acol_ps[:, :], ones64_row[:], blk[0:1, 0:2], start=True, stop=False)
        nc.tensor.matmul(acol_ps[:, :], iota_m32_row[:], beta2[:], start=False, stop=True)

        # ---- D = acol - n per block, squared on DVE (acol read from PSUM)
        nc.vector.tensor_scalar(
            Dsb[:, 128:384], iota256_all[:, :], -1.0, acol_ps[:, 1:2], OP.mult, OP.add
        )
        nc.vector.tensor_scalar(
            Dsb[:, 0:128], iota_dup_all[:, :], -1.0, acol_ps[:, 0:1], OP.mult, OP.add
        )
        nc.vector.tensor_tensor(D2sb[:, 128:384], Dsb[:, 128:384], Dsb[:, 128:384], OP.mult)
        nc.scalar.activation(Ebf[:, 128:384], D2sb[:, 128:384], AF.Exp, scale=-1.0)
        nc.vector.tensor_tensor(D2sb[:, 0:128], Dsb[:, 0:128], Dsb[:, 0:128], OP.mult)
        nc.scalar.activation(Ebf[:, 0:128], D2sb[:, 0:128], AF.Exp, scale=-1.0)
        nc.vector.tensor_copy(wimg_bf[:, 2, :], wimg[:, 2, :])

        # ---- valid, poison, masks (off the D path)
        nc.vector.tensor_scalar(v10[0:1, 0:4], cs[:], 0.0, None, OP.is_ge)
        nc.vector.tensor_scalar(v10[0:1, 4:8], cs[:], 256.0, None, OP.is_le)
        nc.vector.tensor_tensor(v10[0:1, 8:10], cs[0:1, 1:4:2], cs[0:1, 0:3:2], OP.is_gt)
        nc.vector.tensor_reduce(blk[0:1, 8:9], v10[:], AX.X, OP.min)
        nc.vector.tensor_scalar(tmp11[0:1, 0:1], blk[0:1, 8:9], 1e9, -1e9, OP.mult, OP.add)
        nc.vector.tensor_tensor(tmp11[0:1, 1:2], cs[0:1, 1:2], xrow[0:1, 19:20], OP.subtract)
        nc.vector.tensor_tensor(blk[0:1, 5:6], tmp11[0:1, 1:2], tmp11[0:1, 0:1], OP.add)
        nc.vector.tensor_copy(blk[0:1, 6:8], cs[0:1, 2:4])
        nc.vector.tensor_scalar(mtmp[0:1, 0:128], iotadupf[0:1, :], blk[0:1, 4:5], None, OP.is_ge)
        nc.vector.scalar_tensor_tensor(
            mrow[:], iotadupf[0:1, :], blk[0:1, 5:6], mtmp[0:1, 0:128], OP.is_lt, OP.mult
        )
        nc.vector.tensor_scalar(mtmp[0:1, 256:512], iota256f[0:1, :], blk[0:1, 6:7], None, OP.is_ge)
        nc.vector.scalar_tensor_tensor(
            mcol[:], iota256f[0:1, :], blk[0:1, 7:8], mtmp[0:1, 256:512], OP.is_lt, OP.mult
        )
        nc.vector.tensor_copy(mrow_bf[:], mrow[:])
        nc.vector.tensor_scalar(vrow_bf[:], mtmp[0:1, 0:128], 0.0, blk[0:1, 8:9], OP.mult, OP.add)
        nc.vector.tensor_scalar(negcol_bf[:], mcol[:], -1.0, None, OP.mult)

        # ---- PE stream: rowincol, Qv, zcol, zc-cols
        nc.tensor.matmul(zb[:, 1:2], mrow_bf[:], one11_bf[:])
        nc.tensor.matmul(Qv_ps[:, :], vrow_bf[:], ones256_bf[:], start=True, stop=False)
        nc.tensor.matmul(Qv_ps[:, :], mrow_bf[:], negcol_bf[:], start=False, stop=True)
        nc.tensor.matmul(zb[:, 0:1], Ebf[:, 0:128], ones_col64_bf[:])
        nc.tensor.matmul(zc2_ps[:, 0:1], Ebf[:, 128:256], ones_col64_bf[:])
        nc.tensor.matmul(zc2_ps[:, 1:2], Ebf[:, 256:384], ones_col64_bf[:])

        # ---- reciprocals on columns (partition-parallel DVE), then
        # transpose the zc reciprocal back to a row via PE + id128
        nc.vector.tensor_scalar(ztc[:, :], zb[:, 0:1], 1e-30, None, OP.max)
        nc.vector.reciprocal(rcprs[:, :], ztc[:, :])
        nc.vector.tensor_tensor(ssb[:, :], rcprs[:, :], zb[:, 1:2], OP.mult)
        nc.vector.tensor_scalar(zc2t[:, :], zc2_ps[:, 0:2], 1e-30, None, OP.max)
        with nc.allow_low_precision("rzc feeds a bf16 matmul anyway"):
            nc.vector.reciprocal(rcp2_bf[:, :], zc2t[:, :])
        nc.tensor.matmul(zc2_ps[0:1, 4:132], rcp2_bf[:, 0:1], id128[:, :])
        nc.tensor.matmul(zc2_ps[0:1, 132:260], rcp2_bf[:, 1:2], id128[:, :])
        nc.vector.tensor_tensor(scol_bf[:], zc2_ps[0:1, 4:260], mcol[:], OP.mult)

        # ---- T1 matmuls, copies, SCOL + U
        nc.tensor.matmul(T1ps[:, 0:64], wimg_bf[:, 0, :], Ebf[:, 0:64])
        nc.tensor.matmul(T1ps[:, 64:128], wimg_bf[:, 1, :], Ebf[:, 0:64])
        nc.tensor.matmul(T1ps[:, 128:192], wimg_bf[:, 2, :], Ebf[:, 0:64])
        nc.scalar.copy(T1sb[:], T1ps[:, :])
        nc.tensor.matmul(scol_ps[:, :], ones128_row_bf[:], scol_bf[:])
        nc.tensor.matmul(Uc_ps[:, :], T1sb[:, 128:192], Ebf[:, 128:384])
        nc.tensor.matmul(Uab_ps[:, :], T1sb[:, 0:128], Ebf[:, 128:384])
        nc.scalar.copy(Uc_sb[:], Uc_ps[:, :])
        nc.scalar.copy(Uab_sb[:], Uab_ps[:, :])

        # ---- final blend inside the tile (starts at dep-readiness; the
        # tile-exit barrier then orders the bare output DMAs after it)
        nc.vector.scalar_tensor_tensor(
            res_c[:], Uc_sb[:, :], ssb[0:64, 0:1], scol_ps[0:64, :], OP.mult, OP.mult
        )
        nc.vector.tensor_tensor(res_c[:], res_c[:], Qv_ps[0:64, :], OP.add)
        nc.vector.scalar_tensor_tensor(
            res_ab[:], Uab_sb[:, :], ssb[:, 0:1], scol_ps[:, :], OP.mult, OP.mult
        )
        nc.vector.tensor_tensor(res_ab[:], res_ab[:], Qv_ps[:, :], OP.add)

    # ---- post-tile: bare output DMAs on two engines in parallel (the
    # tile-exit all-engine barrier already ordered them after the blend;
    # completion is covered by the ~7us NRT postamble + queue quiesce)
    semD = nc.alloc_semaphore("outD")
    semE = nc.alloc_semaphore("outE")
    nc.scalar.dma_start(out_d[2, :, :], res_c[:]).then_inc(semE, 16)
    nc.sync.dma_start(
        out_d[0:2, :, :].rearrange("a b c -> (a b) c"), res_ab[:]
    ).then_inc(semD, 16)

    nc.compile()
    return nc


_CACHE = {}


def get_nc():
    if "nc" not in _CACHE:
        _CACHE["nc"] = build_nc()
    return _CACHE["nc"]


def make_in_maps(X, images):
    X = np.ascontiguousarray(np.asarray(X, np.float32))
    images = np.ascontiguousarray(np.asarray(images, np.float32))
    in_maps = []
    for c in range(N_CORES):
        pic, rb = divmod(c, 4)
        xm = np.zeros((1, 20), np.float32)
        xm[0, :19] = X[pic, 0]
        xm[0, 19] = float(RB * rb)
        in_maps.append({"xmeta": xm, "images": images})
    return in_maps


def assemble(results):
    out = np.empty((2, 3, H, H), np.float32)
    for c in range(N_CORES):
        pic, rb = divmod(c, 4)
        out[pic, :, RB * rb : RB * (rb + 1), :] = results[c]["out"]
    return out


def _axon_reset():
    try:
        import ctypes

        import jax

        jax.devices()
        ctypes.CDLL("/opt/axon/libaxon_pjrt.so").axon_reset()
    except Exception:
        pass


def kernel(X, images):
    nc = get_nc()
    in_maps = make_in_maps(X, images)
    try:
        res = run_bass_kernel_spmd(nc, in_maps, list(range(N_CORES)))
    except Exception:
        # the axon terminal can be left in a bad state by earlier failed
        # runs (LoadExecutable errors); reset and retry once
        _axon_reset()
        res = run_bass_kernel_spmd(nc, in_maps, list(range(N_CORES)))
    return assemble(res.results)


# revision 31
# speedup vs baseline: 1.0721x; 1.0721x over previous
"""Trainium2 Bass kernel for the emoji-box decoder problem (v2).

Math: softmax(-d2) over emoji pixels is separable (rows x cols).  Per
core (64 canvas rows x 256 cols x 3ch of one picture):

  E = exp(-D2),  D2 built by ONE PE matmul with lhsT=[1, p-32, (p-32)^2]
  and rhs rows [u^2; 2*beta*u; beta^2] (u = gamma - n), giving
    E[:, 0:128]   = erT[i, (h,r)]  (row kernel, transposed, h-duplicated)
    E[:, 128:384] = ecT[j, c]      (col kernel, transposed)
  T1_ch[j, r] = sum_i img[ch,i,j] * erT[i,r]          (3 matmuls)
  U[(ch,r), c] = sum_j T1[j,(ch,r)] * ecT[j,c]        (2 matmuls)
  res = U * srow[(ch,r)] * SCOL[c] + Qv               (2 DVE ops/piece)
  with srow = rzr*rowin (transposed via PE), SCOL = ones (x) (rzc*colin),
  Qv = valid - rowin (x) colin (PE outer products).

No max-shift in the softmax: in-box distances are <= ~16 so exp(-d2)
stays in range; fully-underflowed rows are saved by a 1e-30 clamp on
the denominator and are masked anyway.

The final blend + output DMAs run OUTSIDE the TileContext so the fixed
NRT postamble (a ~6us per-engine semaphore-clear storm, longest on the
Tensor engine) overlaps the output-DMA tail instead of following it.

Sharding: 8 cores = 2 pictures x 4 row-blocks of 64 canvas rows; images
replicated; xmeta = [X row (19), r0].
"""

import sys

import numpy as np

if "/opt/trn_rl_repo" not in sys.path:
    sys.path.insert(0, "/opt/trn_rl_repo")

import concourse.bacc as bacc
import concourse.bass as bass
import concourse.mybir as mybir
import concourse.tile as tile
from concourse.bass_utils import run_bass_kernel_spmd


def _ensure_ntff_hook():
    """The image's antenv package lacks axon_hooks, so trn_boot's NTFF
    profile hook install degrades silently and run_bass_kernel_spmd
    crashes on `from antenv.axon_hooks import ...` when trace=True.
    Provide the module and install the ctypes hook ourselves."""
    import types

    try:
        from antenv.axon_hooks import get_axon_ntff_profile_hook  # noqa: F401

        return
    except ImportError:
        pass
    mod = types.ModuleType("antenv.axon_hooks")
    _hook = [None]
    mod.set_axon_ntff_profile_hook = lambda h: _hook.__setitem__(0, h)
    mod.get_axon_ntff_profile_hook = lambda: _hook[0]
    try:
        import antenv

        sys.modules["antenv.axon_hooks"] = mod
        antenv.axon_hooks = mod
        from trn_agent_boot.trn_boot import _ntff_profile_via_ctypes

        hook = _ntff_profile_via_ctypes("/opt/axon/libaxon_pjrt.so")
        if hook is not None:
            mod.set_axon_ntff_profile_hook(hook)
    except Exception:
        pass


_ensure_ntff_hook()

F32 = mybir.dt.float32
BF = mybir.dt.bfloat16
I32 = mybir.dt.int32
U32 = mybir.dt.uint32
AF = mybir.ActivationFunctionType
OP = mybir.AluOpType
AX = mybir.AxisListType

MAGIC = 8388608.0  # 2**23; x + MAGIC - MAGIC == rint(x) for 0 <= x < 2**22

N_CORES = 8
H = 256
S = 64
N_IMG = 14
RB = 64  # canvas rows per core


def build_nc():
    nc = bacc.Bacc("TRN2", target_bir_lowering=False, debug=False)

    xmeta_d = nc.dram_tensor("xmeta", [1, 20], F32, kind="ExternalInput")
    images_d = nc.dram_tensor("images", [N_IMG, 4, S, S], F32, kind="ExternalInput")
    out_d = nc.dram_tensor("out", [3, RB, H], F32, kind="ExternalOutput")

    # ---- raw allocations (persist across the TileContext boundary) ----
    sb = lambda name, shape, dt=F32: nc.alloc_sbuf_tensor(name, shape, dt)
    xrow = sb("xrow", [1, 20])
    iota256f = sb("iota256f", [1, 256])
    iotadupf = sb("iotadupf", [1, 128])      # [0..63, 0..63]
    iota_m32_row = sb("iota_m32_row", [1, 64])   # p-32 as a row (acol matmul)
    ones64_row = sb("ones64_row", [1, 64])
    iota_dup_all = sb("iota_dup_all", [64, 128])   # [0..63,0..63] every partition
    iota256_all = sb("iota256_all", [64, 256])     # 0..255 every partition
    ones_col64_bf = sb("ones_col64_bf", [64, 1], BF)
    ones128_row_bf = sb("ones128_row_bf", [1, 128], BF)
    ones256_bf = sb("ones256_bf", [1, 256], BF)
    one11_bf = sb("one11_bf", [1, 1], BF)
    blk = sb("blk", [1, 12])                 # p0 scalars
    cs = sb("cs", [1, 4])
    box2 = sb("box2", [1, 2])
    beta2 = sb("beta2", [1, 2])
    v10 = sb("v10", [1, 10])
    tmp11 = sb("tmp11", [1, 2])
    acol = sb("acol", [64, 2])               # a_r, a_c per-partition
    Dsb = sb("Dsb", [64, 384])
    D2sb = sb("D2sb", [64, 384])
    E = sb("E", [64, 384])
    Ebf = sb("Ebf", [64, 384], BF)
    wimg = sb("wimg", [64, 3, 64])
    wimg_bf = sb("wimg_bf", [64, 3, 64], BF)
    T1sb = sb("T1sb", [64, 192], BF)
    mrow = sb("mrow", [1, 128])
    mcol = sb("mcol", [1, 256])
    mrow_bf = sb("mrow_bf", [1, 128], BF)
    mtmp = sb("mtmp", [1, 512])
    rcprs = sb("rcprs", [128, 1])
    ztc = sb("ztc", [128, 1])
    zc2t = sb("zc2t", [128, 2])
    rcp2 = sb("rcp2", [128, 2])
    id128 = sb("id128", [128, 128], BF)
    ones128sq = sb("ones128sq", [128, 128], BF)
    rcp2_bf = sb("rcp2_bf", [128, 2], BF)
    rsqr = sb("rsqr", [1, 256])              # rsqrt(zrow)
    rzc = sb("rzc", [1, 256])
    scol = sb("scol", [1, 256])
    scol_bf = sb("scol_bf", [1, 256], BF)
    Uab_sb = sb("Uab_sb", [128, 256])
    Uc_sb = sb("Uc_sb", [64, 256])
    ssb = sb("ssb", [128, 1])
    vrow_bf = sb("vrow_bf", [1, 128], BF)
    negcol_bf = sb("negcol_bf", [1, 256], BF)
    mx8 = sb("mx8", [1, 8])
    idx8 = sb("idx8", [1, 8], U32)
    res_ab = sb("res_ab", [128, 256])
    res_c = sb("res_c", [64, 256])
    eps128 = sb("eps128", [128, 1])
    eps1 = sb("eps1", [1, 1])
    warm1 = sb("warm1", [1, 1])
    warm2 = sb("warm2", [1, 1])

    ps = lambda name, shape: nc.alloc_psum_tensor(name, shape)
    zb = ps("zb", [128, 512])
    zc2_ps = ps("zc2_ps", [128, 512])  # zc cols [128,0:2]; rcp rows [0:1,4:260]
    acol_ps = ps("acol_ps", [64, 2])     # zcol [128,0:1], rowincol [128,1:2], zrow [0:1,2:258]
    T1ps = ps("T1ps", [64, 192])
    Uab_ps = ps("Uab_ps", [128, 256])
    Uc_ps = ps("Uc_ps", [64, 256])
    scol_ps = ps("scol_ps", [128, 256])
    Qv_ps = ps("Qv_ps", [128, 256])

    with tile.TileContext(nc) as tc:  # noqa: F841
        # ---- warm the scalar-engine Exp table early (overlaps input DMA)
        nc.gpsimd.memset(warm1[:], 0.0)
        nc.scalar.activation(warm2[:], warm1[:], AF.Exp)

        # ---- input DMA first
        nc.sync.dma_start(xrow[:], xmeta_d[:])

        # ---- emoji index first: top-1 via Max8 + MaxIndex, straight off xrow
        nc.vector.max(mx8[:], xrow[0:1, 5:19])
        nc.vector.max_index(idx8[:], mx8[:], xrow[0:1, 5:19])
        with nc.gpsimd.register("ridx") as ridx:
            nc.gpsimd.reg_load(ridx, idx8[0:1, 0:1])
            off = nc.gpsimd.snap(ridx)
            nc.gpsimd.dma_start(
                wimg_bf[:, 0:2, :],
                images_d[bass.ds(off, 1), 0:2, :, :].squeeze(0).transpose([1, 0, 2]),
            )
        with nc.sync.register("ridx2") as ridx2:
            nc.sync.reg_load(ridx2, idx8[0:1, 0:1])
            off2 = nc.sync.snap(ridx2)
            nc.sync.dma_start(
                wimg[:, 2, :],
                images_d[bass.ds(off2, 1), 2, :, :].squeeze(0),
            )

        # ---- constants (no data deps): f32 iotas are exact for values < 2^24
        IOTA = dict(allow_small_or_imprecise_dtypes=True)
        nc.gpsimd.iota(iota256f[0:1, :], pattern=[[1, 256]], base=0, channel_multiplier=0, **IOTA)
        nc.gpsimd.iota(iotadupf[0:1, :], pattern=[[0, 2], [1, 64]], base=0, channel_multiplier=0, **IOTA)
        nc.gpsimd.iota(iota_m32_row[0:1, :], pattern=[[1, 64]], base=-32, channel_multiplier=0, **IOTA)
        nc.gpsimd.iota(iota_dup_all[:, :], pattern=[[0, 2], [1, 64]], base=0, channel_multiplier=0, **IOTA)
        nc.gpsimd.iota(iota256_all[:, :], pattern=[[1, 256]], base=0, channel_multiplier=0, **IOTA)
        nc.vector.memset(ones64_row[:], 1.0)
        nc.gpsimd.memset(ones_col64_bf[:], 1.0)
        nc.vector.memset(ones128_row_bf[:], 1.0)
        nc.vector.memset(ones256_bf[:], 1.0)
        nc.vector.memset(one11_bf[:], 1.0)
        nc.vector.memset(eps128[:], 1e-30)
        nc.gpsimd.memset(ones128sq[:], 1.0)
        nc.gpsimd.affine_select(
            id128[:], ones128sq[:], pattern=[[1, 128]], compare_op=OP.is_equal,
            fill=0.0, base=0, channel_multiplier=-1,
        )

        # ---- p0 scalar chain -> blk = [gr, gc, br, bc, x1r, x2rp, y1, y2, valid]
        nc.vector.tensor_scalar(cs[:], xrow[0:1, 0:4], 256.0, MAGIC, OP.mult, OP.add)
        nc.vector.tensor_scalar(cs[:], cs[:], MAGIC, None, OP.subtract)
        nc.vector.tensor_tensor(box2[:], cs[0:1, 1:4:2], cs[0:1, 0:3:2], OP.subtract)
        nc.vector.tensor_scalar(beta2[:], box2[:], 1.0 / 64.0, None, OP.mult)
        nc.vector.tensor_tensor(blk[0:1, 4:5], cs[0:1, 0:1], xrow[0:1, 19:20], OP.subtract)
        nc.vector.scalar_tensor_tensor(
            blk[0:1, 0:1], beta2[0:1, 0:1], 32.0, blk[0:1, 4:5], OP.mult, OP.add
        )
        nc.vector.scalar_tensor_tensor(
            blk[0:1, 1:2], beta2[0:1, 1:2], 32.0, cs[0:1, 2:3], OP.mult, OP.add
        )
        # ---- acol[p, :] = [gamma_r, gamma_c] + (p-32)*[beta_r, beta_c]
        # via two accumulating rank-1 PE matmuls (PE is idle this early)
        nc.tensor.matmul(acol_ps[:, :], ones64_row[:], blk[0:1, 0:2], start=True, stop=False)
        nc.tensor.matmul(acol_ps[:, :], iota_m32_row[:], beta2[:], start=False, stop=True)

        # ---- D = acol - n per block, squared on DVE (acol read from PSUM)
        nc.vector.tensor_scalar(
            Dsb[:, 128:384], iota256_all[:, :], -1.0, acol_ps[:, 1:2], OP.mult, OP.add
        )
        nc.vector.tensor_scalar(
            Dsb[:, 0:128], iota_dup_all[:, :], -1.0, acol_ps[:, 0:1], OP.mult, OP.add
        )
        nc.vector.tensor_tensor(D2sb[:, 128:384], Dsb[:, 128:384], Dsb[:, 128:384], OP.mult)
        nc.scalar.activation(Ebf[:, 128:384], D2sb[:, 128:384], AF.Exp, scale=-1.0)
        nc.vector.tensor_tensor(D2sb[:, 0:128], Dsb[:, 0:128], Dsb[:, 0:128], OP.mult)
        nc.scalar.activation(Ebf[:, 0:128], D2sb[:, 0:128], AF.Exp, scale=-1.0)
        nc.vector.tensor_copy(wimg_bf[:, 2, :], wimg[:, 2, :])

        # ---- valid, poison, masks (off the D path)
        nc.vector.tensor_scalar(v10[0:1, 0:4], cs[:], 0.0, None, OP.is_ge)
        nc.vector.tensor_scalar(v10[0:1, 4:8], cs[:], 256.0, None, OP.is_le)
        nc.vector.tensor_tensor(v10[0:1, 8:10], cs[0:1, 1:4:2], cs[0:1, 0:3:2], OP.is_gt)
        nc.vector.tensor_reduce(blk[0:1, 8:9], v10[:], AX.X, OP.min)
        nc.vector.tensor_scalar(tmp11[0:1, 0:1], blk[0:1, 8:9], 1e9, -1e9, OP.mult, OP.add)
        nc.vector.tensor_tensor(tmp11[0:1, 1:2], cs[0:1, 1:2], xrow[0:1, 19:20], OP.subtract)
        nc.vector.tensor_tensor(blk[0:1, 5:6], tmp11[0:1, 1:2], tmp11[0:1, 0:1], OP.add)
        nc.vector.tensor_copy(blk[0:1, 6:8], cs[0:1, 2:4])
        nc.vector.tensor_scalar(mtmp[0:1, 0:128], iotadupf[0:1, :], blk[0:1, 4:5], None, OP.is_ge)
        nc.vector.scalar_tensor_tensor(
            mrow[:], iotadupf[0:1, :], blk[0:1, 5:6], mtmp[0:1, 0:128], OP.is_lt, OP.mult
        )
        nc.vector.tensor_scalar(mtmp[0:1, 256:512], iota256f[0:1, :], blk[0:1, 6:7], None, OP.is_ge)
        nc.vector.scalar_tensor_tensor(
            mcol[:], iota256f[0:1, :], blk[0:1, 7:8], mtmp[0:1, 256:512], OP.is_lt, OP.mult
        )
        nc.vector.tensor_copy(mrow_bf[:], mrow[:])
        nc.vector.tensor_scalar(vrow_bf[:], mtmp[0:1, 0:128], 0.0, blk[0:1, 8:9], OP.mult, OP.add)
        nc.vector.tensor_scalar(negcol_bf[:], mcol[:], -1.0, None, OP.mult)

        # ---- PE stream: rowincol, Qv, zcol, zc-cols
        nc.tensor.matmul(zb[:, 1:2], mrow_bf[:], one11_bf[:])
        nc.tensor.matmul(Qv_ps[:, :], vrow_bf[:], ones256_bf[:], start=True, stop=False)
        nc.tensor.matmul(Qv_ps[:, :], mrow_bf[:], negcol_bf[:], start=False, stop=True)
        nc.tensor.matmul(zb[:, 0:1], Ebf[:, 0:128], ones_col64_bf[:])
        nc.tensor.matmul(zc2_ps[:, 0:1], Ebf[:, 128:256], ones_col64_bf[:])
        nc.tensor.matmul(zc2_ps[:, 1:2], Ebf[:, 256:384], ones_col64_bf[:])

        # ---- reciprocals on columns (partition-parallel DVE), then
        # transpose the zc reciprocal back to a row via PE + id128
        nc.vector.tensor_scalar(ztc[:, :], zb[:, 0:1], 1e-30, None, OP.max)
        nc.vector.reciprocal(rcprs[:, :], ztc[:, :])
        nc.vector.tensor_tensor(ssb[:, :], rcprs[:, :], zb[:, 1:2], OP.mult)
        nc.vector.tensor_scalar(zc2t[:, :], zc2_ps[:, 0:2], 1e-30, None, OP.max)
        with nc.allow_low_precision("rzc feeds a bf16 matmul anyway"):
            nc.vector.reciprocal(rcp2_bf[:, :], zc2t[:, :])
        nc.tensor.matmul(zc2_ps[0:1, 4:132], rcp2_bf[:, 0:1], id128[:, :])
        nc.tensor.matmul(zc2_ps[0:1, 132:260], rcp2_bf[:, 1:2], id128[:, :])
        nc.vector.tensor_tensor(scol_bf[:], zc2_ps[0:1, 4:260], mcol[:], OP.mult)

        # ---- T1 matmuls, copies, SCOL + U
        nc.tensor.matmul(T1ps[:, 0:64], wimg_bf[:, 0, :], Ebf[:, 0:64])
        nc.tensor.matmul(T1ps[:, 64:128], wimg_bf[:, 1, :], Ebf[:, 0:64])
        nc.tensor.matmul(T1ps[:, 128:192], wimg_bf[:, 2, :], Ebf[:, 0:64])
        nc.scalar.copy(T1sb[:], T1ps[:, :])
        nc.tensor.matmul(scol_ps[:, :], ones128_row_bf[:], scol_bf[:])
        nc.tensor.matmul(Uc_ps[:, :], T1sb[:, 128:192], Ebf[:, 128:384])
        nc.tensor.matmul(Uab_ps[:, :], T1sb[:, 0:128], Ebf[:, 128:384])
        nc.scalar.copy(Uc_sb[:], Uc_ps[:, :])
        nc.scalar.copy(Uab_sb[:], Uab_ps[:, :])

    # ---- post-tile: blend + output DMAs (engines end as early as possible;
    # res_c first since U_c finishes first)
    semA = nc.alloc_semaphore("postA")
    semB = nc.alloc_semaphore("postB")
    semD = nc.alloc_semaphore("postD")
    semE = nc.alloc_semaphore("postE")

    nc.vector.scalar_tensor_tensor(
        res_c[:], Uc_sb[:, :], ssb[0:64, 0:1], scol_ps[0:64, :], OP.mult, OP.mult
    )
    nc.vector.drain()
    nc.vector.tensor_tensor(res_c[:], res_c[:], Qv_ps[0:64, :], OP.add).then_inc(semB)
    nc.vector.scalar_tensor_tensor(
        res_ab[:], Uab_sb[:, :], ssb[:, 0:1], scol_ps[:, :], OP.mult, OP.mult
    )
    nc.vector.drain()
    nc.vector.tensor_tensor(res_ab[:], res_ab[:], Qv_ps[:, :], OP.add).then_inc(semA)

    nc.scalar.wait_ge(semB, 1)
    nc.scalar.dma_start(out_d[2, :, :], res_c[:]).then_inc(semE, 16)
    nc.sync.wait_ge(semA, 1)
    nc.sync.dma_start(
        out_d[0:2, :, :].rearrange("a b c -> (a b) c"), res_ab[:]
    ).then_inc(semD, 16)
    # No explicit completion wait: the ~7us NRT postamble (semaphore-clear
    # storm + ring barriers) runs after these streams end, giving the
    # in-flight output DMAs >4x time margin before NEFF completion, and
    # NRT quiesces DMA queues at teardown.

    nc.compile()
    return nc


_CACHE = {}


def get_nc():
    if "nc" not in _CACHE:
        _CACHE["nc"] = build_nc()
    return _CACHE["nc"]


def make_in_maps(X, images):
    X = np.ascontiguousarray(np.asarray(X, np.float32))
    images = np.ascontiguousarray(np.asarray(images, np.float32))
    in_maps = []
    for c in range(N_CORES):
        pic, rb = divmod(c, 4)
        xm = np.zeros((1, 20), np.float32)
        xm[0, :19] = X[pic, 0]
        xm[0, 19] = float(RB * rb)
        in_maps.append({"xmeta": xm, "images": images})
    return in_maps


def assemble(results):
    out = np.empty((2, 3, H, H), np.float32)
    for c in range(N_CORES):
        pic, rb = divmod(c, 4)
        out[pic, :, RB * rb : RB * (rb + 1), :] = results[c]["out"]
    return out


def _axon_reset():
    try:
        import ctypes

        import jax

        jax.devices()
        ctypes.CDLL("/opt/axon/libaxon_pjrt.so").axon_reset()
    except Exception:
        pass


def kernel(X, images):
    nc = get_nc()
    in_maps = make_in_maps(X, images)
    try:
        res = run_bass_kernel_spmd(nc, in_maps, list(range(N_CORES)))
    except Exception:
        # the axon terminal can be left in a bad state by earlier failed
        # runs (LoadExecutable errors); reset and retry once
        _axon_reset()
        res = run_bass_kernel_spmd(nc, in_maps, list(range(N_CORES)))
    return assemble(res.results)
